# revision 4
# baseline (speedup 1.0000x reference)
"""Trainium2 Bass kernel for nn_LocalGroupedZernikeNewBP.

Full inputs in, full output out. Shards raw_coeffs [8,256,256,36] along the
batch dim: one image per NeuronCore (8 cores). Scalar params are baked into
the compiled program as immediates (rebuilt per distinct param values).

Per-core program (image [H=256, W=256, C=36], HWC contiguous):
  - 2 row-blocks of 128 rows; SBUF tile [128, W*C], DMAs fully contiguous.
  - special (ch 0:3):  out = amax * tanh(alpha*(x+bias))
  - joint groups low(3:6) mid(6:15) high(15:36):
      soft_abs = sqrt((x+bias)^2 + eps)         (ACT Square, ACT Sqrt)
      chansum  = sum_c soft_abs                 (DVE reduce over C)
      s        = 3x3 box(chansum), edge-replicated (DVE shifted adds,
                 halo row exchanged between the two row-blocks)
      gain     = alpha*gss / (1 + s/p_sat)      (DVE)
      v        = (x+bias) * gain                (DVE scalar_tensor_tensor,
                                                 gain broadcast over C)
      out      = amax * tanh(v)                 (ACT Tanh whole tile,
                                                 GPSIMD amax mul)
"""

import numpy as np

B, H, W, C = 8, 256, 256, 36
FD = W * C  # 9216 free elements per row
GROUPS = [("low", 3, 6), ("mid", 6, 15), ("high", 15, 36)]
N_CORES = 8

_NC_CACHE: dict[tuple, object] = {}


def _build(p: dict[str, float]):
    from contextlib import ExitStack

    import concourse.bass as bass  # noqa: F401
    import concourse.tile as tile
    from concourse import bacc, mybir

    f32 = mybir.dt.float32
    AF = mybir.ActivationFunctionType
    OP = mybir.AluOpType
    AX = mybir.AxisListType

    nc = bacc.Bacc(
        "TRN2", target_bir_lowering=False, debug=False, num_devices=N_CORES
    )
    x = nc.dram_tensor("x", [H, FD], f32, kind="ExternalInput").ap()
    y = nc.dram_tensor("y", [H, FD], f32, kind="ExternalOutput").ap()

    with tile.TileContext(nc) as tc, ExitStack() as ctx:
        xp = ctx.enter_context(tc.tile_pool(name="xp", bufs=2))
        sqp = ctx.enter_context(tc.tile_pool(name="sqp", bufs=2))
        mp = ctx.enter_context(tc.tile_pool(name="maps", bufs=1))
        cp = ctx.enter_context(tc.tile_pool(name="consts", bufs=1))

        _consts: dict[float, object] = {}

        def constant(val: float):
            """[128,1] SBUF tile holding `val` (for ACT bias operands)."""
            val = float(val)
            if val not in _consts:
                ct = cp.tile([128, 1], f32, tag=f"const{len(_consts)}")
                nc.vector.memset(ct[:], val)
                _consts[val] = ct
            return _consts[val][:]

        xt = []
        Tm = {}
        Sm = {}

        # Phase A: load, soft_abs, channel sums, W-direction box -> T maps
        for rb in range(2):
            t = xp.tile([128, FD], f32, tag="x")
            nc.sync.dma_start(t[:], x[rb * 128 : (rb + 1) * 128, :])
            xt.append(t)
            x3 = t[:].rearrange("p (w c) -> p w c", c=C)
            for g, c0, c1 in GROUPS:
                cg = c1 - c0
                sq = sqp.tile([128, W * cg], f32, tag="sq")
                sq3 = sq[:].rearrange("p (w c) -> p w c", c=cg)
                nc.scalar.activation(sq3, x3[:, :, c0:c1], AF.Square,
                                     bias=constant(p[g + "_bias"]))
                nc.scalar.activation(sq[:], sq[:], AF.Sqrt,
                                     bias=constant(p[g + "_eps"]))
                cs = mp.tile([128, W], f32, tag=f"cs{rb}{g}")
                nc.vector.reduce_sum(cs[:], sq3, axis=AX.X)
                # W-direction 3-tap box with replicate edges
                T = mp.tile([128, W], f32, tag=f"T{rb}{g}")
                nc.vector.tensor_add(T[:, 1 : W - 1], cs[:, 0 : W - 2],
                                     cs[:, 2:W])
                nc.vector.tensor_add(T[:, 0:1], cs[:, 0:1], cs[:, 1:2])
                nc.vector.tensor_add(T[:, W - 1 : W], cs[:, W - 2 : W - 1],
                                     cs[:, W - 1 : W])
                nc.vector.tensor_add(T[:], T[:], cs[:])
                Tm[(rb, g)] = T

        # Phase A2: H-direction 3-tap box (halo row from the other block).
        # Compute-engine SBUF APs must start at partition 0/32/64/96, so the
        # +-1 row shifts are done with SBUF->SBUF DMA copies, then aligned
        # adds. Then gain map G = alpha*gss / (1 + s/p_sat), in-place.
        for rb in range(2):
            for g, c0, c1 in GROUPS:
                T = Tm[(rb, g)]
                To = Tm[(1 - rb, g)]
                U = mp.tile([128, W], f32, tag=f"U{rb}{g}")
                D = mp.tile([128, W], f32, tag=f"D{rb}{g}")
                nc.sync.dma_start(U[1:128, :], T[0:127, :])
                nc.sync.dma_start(D[0:127, :], T[1:128, :])
                if rb == 0:
                    nc.sync.dma_start(U[0:1, :], T[0:1, :])
                    nc.sync.dma_start(D[127:128, :], To[0:1, :])
                else:
                    nc.sync.dma_start(U[0:1, :], To[127:128, :])
                    nc.sync.dma_start(D[127:128, :], T[127:128, :])
                S = mp.tile([128, W], f32, tag=f"S{rb}{g}")
                nc.vector.tensor_add(S[:], T[:], U[:])
                nc.vector.tensor_add(S[:], S[:], D[:])
                nc.vector.tensor_scalar(S[:], S[:],
                                        float(1.0 / p[g + "_p_sat"]), 1.0,
                                        op0=OP.mult, op1=OP.add)
                nc.vector.reciprocal(S[:], S[:])
                nc.vector.tensor_scalar_mul(
                    S[:], S[:], float(p[g + "_alpha"] * p[g + "_gss"]))
                Sm[(rb, g)] = S

        # Phase B: v = (x+bias)*gain in place, tanh, amax, store
        for rb in range(2):
            t = xt[rb]
            x3 = t[:].rearrange("p (w c) -> p w c", c=C)
            sp = x3[:, :, 0:3]
            nc.vector.tensor_scalar(
                sp, sp, float(p["special_alpha"]),
                float(p["special_alpha"] * p["special_bias"]),
                op0=OP.mult, op1=OP.add)
            for g, c0, c1 in GROUPS:
                cg = c1 - c0
                gb = Sm[(rb, g)][:].unsqueeze(2).to_broadcast([128, W, cg])
                nc.vector.scalar_tensor_tensor(
                    x3[:, :, c0:c1], x3[:, :, c0:c1],
                    float(p[g + "_bias"]), gb, op0=OP.add, op1=OP.mult)
            nc.scalar.activation(t[:], t[:], AF.Tanh)
            nc.gpsimd.tensor_scalar_mul(x3[:, :, 0:3], x3[:, :, 0:3],
                                        float(p["special_amax"]))
            for g, c0, c1 in GROUPS:
                nc.gpsimd.tensor_scalar_mul(x3[:, :, c0:c1], x3[:, :, c0:c1],
                                            float(p[g + "_amax"]))
            nc.sync.dma_start(y[rb * 128 : (rb + 1) * 128, :], t[:])

    nc.compile()
    return nc


_SCALARS = [
    "special_bias", "special_alpha", "special_amax", "special_eps",
    "low_bias", "low_alpha", "low_amax", "low_eps", "low_gss", "low_p_sat",
    "mid_bias", "mid_alpha", "mid_amax", "mid_eps", "mid_gss", "mid_p_sat",
    "high_bias", "high_alpha", "high_amax", "high_eps", "high_gss",
    "high_p_sat",
]


def build_nc(**inputs):
    """Build (or fetch cached) compiled Bass program for these scalar params."""
    p = {k: float(np.asarray(inputs[k]).reshape(-1)[0]) for k in _SCALARS}
    key = tuple(p[k] for k in _SCALARS)
    if key not in _NC_CACHE:
        _NC_CACHE[key] = _build(p)
    return _NC_CACHE[key]


def kernel(**inputs) -> np.ndarray:
    from concourse.bass_utils import run_bass_kernel_spmd

    raw = np.ascontiguousarray(np.asarray(inputs["raw_coeffs"],
                                          dtype=np.float32))
    assert raw.shape == (B, H, W, C), raw.shape
    nc = build_nc(**inputs)
    in_maps = [{"x": raw[i].reshape(H, FD)} for i in range(N_CORES)]
    res = run_bass_kernel_spmd(nc, in_maps, list(range(N_CORES)))
    out = np.stack([res.results[i]["y"].reshape(H, W, C)
                    for i in range(N_CORES)])
    return out.astype(np.float32)


# revision 6
# speedup vs baseline: 2.3503x; 2.3503x over previous
"""Trainium2 Bass kernel for nn_LocalGroupedZernikeNewBP.

Full inputs in, full output out. Shards raw_coeffs [8,256,256,36] along the
batch dim: one image per NeuronCore (8 cores). Scalar params are baked into
the compiled program as immediates (rebuilt per distinct param values).

Per-core program (image [H=256, W=256, C=36], HWC contiguous):
  - 2 row-blocks of 128 rows; SBUF tile [128, W*C], DMAs fully contiguous.
  - special (ch 0:3):  out = amax * tanh(alpha*(x+bias))
  - joint groups low(3:6) mid(6:15) high(15:36):
      soft_abs = sqrt((x+bias)^2 + eps)         (ACT Square, ACT Sqrt)
      chansum  = sum_c soft_abs                 (DVE reduce over C)
      s        = 3x3 box(chansum), edge-replicated (DVE shifted adds,
                 halo row exchanged between the two row-blocks)
      gain     = alpha*gss / (1 + s/p_sat)      (DVE)
      v        = (x+bias) * gain                (DVE scalar_tensor_tensor,
                                                 gain broadcast over C)
      out      = amax * tanh(v)                 (ACT Tanh whole tile,
                                                 GPSIMD amax mul)
"""

import numpy as np

B, H, W, C = 8, 256, 256, 36
FD = W * C  # 9216 free elements per row
GROUPS = [("low", 3, 6), ("mid", 6, 15), ("high", 15, 36)]
N_CORES = 8

_NC_CACHE: dict[tuple, object] = {}


def _build(p: dict[str, float]):
    from contextlib import ExitStack

    import concourse.bass as bass  # noqa: F401
    import concourse.tile as tile
    from concourse import bacc, mybir

    f32 = mybir.dt.float32
    AF = mybir.ActivationFunctionType
    OP = mybir.AluOpType
    AX = mybir.AxisListType

    nc = bacc.Bacc(
        "TRN2", target_bir_lowering=False, debug=False, num_devices=N_CORES
    )
    x = nc.dram_tensor("x", [H, FD], f32, kind="ExternalInput").ap()
    y = nc.dram_tensor("y", [H, FD], f32, kind="ExternalOutput").ap()

    with tile.TileContext(nc) as tc, ExitStack() as ctx:
        xp = ctx.enter_context(tc.tile_pool(name="xp", bufs=2))
        sqp = ctx.enter_context(tc.tile_pool(name="sqp", bufs=2))
        mp = ctx.enter_context(tc.tile_pool(name="maps", bufs=1))
        cp = ctx.enter_context(tc.tile_pool(name="consts", bufs=1))

        _consts: dict[float, object] = {}

        def constant(val: float):
            """[128,1] SBUF tile holding `val` (for ACT bias operands)."""
            val = float(val)
            if val not in _consts:
                ct = cp.tile([128, 1], f32, tag=f"const{len(_consts)}")
                nc.vector.memset(ct[:], val)
                _consts[val] = ct
            return _consts[val][:]

        xt = []
        Tm = {}
        Sm = {}

        # Phase A: load, soft_abs, channel sums, W-direction box -> T maps
        for rb in range(2):
            t = xp.tile([128, FD], f32, tag="x")
            nc.sync.dma_start(t[:], x[rb * 128 : (rb + 1) * 128, :])
            xt.append(t)
            x3 = t[:].rearrange("p (w c) -> p w c", c=C)
            for g, c0, c1 in GROUPS:
                cg = c1 - c0
                sq = sqp.tile([128, W * cg], f32, tag="sq")
                sq3 = sq[:].rearrange("p (w c) -> p w c", c=cg)
                nc.scalar.activation(sq3, x3[:, :, c0:c1], AF.Square,
                                     bias=constant(p[g + "_bias"]))
                nc.scalar.activation(sq[:], sq[:], AF.Sqrt,
                                     bias=constant(p[g + "_eps"]))
                cs = mp.tile([128, W], f32, tag=f"cs{rb}{g}")
                nc.vector.reduce_sum(cs[:], sq3, axis=AX.X)
                # W-direction 3-tap box with replicate edges
                T = mp.tile([128, W], f32, tag=f"T{rb}{g}")
                nc.vector.tensor_add(T[:, 1 : W - 1], cs[:, 0 : W - 2],
                                     cs[:, 2:W])
                nc.vector.tensor_add(T[:, 0:1], cs[:, 0:1], cs[:, 1:2])
                nc.vector.tensor_add(T[:, W - 1 : W], cs[:, W - 2 : W - 1],
                                     cs[:, W - 1 : W])
                nc.vector.tensor_add(T[:], T[:], cs[:])
                Tm[(rb, g)] = T

        # Phase A2: H-direction 3-tap box (halo row from the other block).
        # Compute-engine SBUF APs must start at partition 0/32/64/96, so the
        # +-1 row shifts are done with SBUF->SBUF DMA copies, then aligned
        # adds. The three group maps of a block share one tile so one
        # reciprocal covers them: G = 1/(s/(p_sat*alpha*gss) + 1/(alpha*gss))
        # = alpha*gss/(1 + s/p_sat).
        for rb in range(2):
            Sall = mp.tile([128, 3 * W], f32, tag=f"Sall{rb}")
            for gi, (g, c0, c1) in enumerate(GROUPS):
                T = Tm[(rb, g)]
                To = Tm[(1 - rb, g)]
                U = mp.tile([128, W], f32, tag=f"U{rb}{g}")
                D = mp.tile([128, W], f32, tag=f"D{rb}{g}")
                nc.sync.dma_start(U[1:128, :], T[0:127, :])
                nc.sync.dma_start(D[0:127, :], T[1:128, :])
                if rb == 0:
                    nc.sync.dma_start(U[0:1, :], T[0:1, :])
                    nc.sync.dma_start(D[127:128, :], To[0:1, :])
                else:
                    nc.sync.dma_start(U[0:1, :], To[127:128, :])
                    nc.sync.dma_start(D[127:128, :], T[127:128, :])
                S = Sall[:, gi * W : (gi + 1) * W]
                nc.vector.tensor_add(S, T[:], U[:])
                nc.vector.tensor_add(S, S, D[:])
                ags = p[g + "_alpha"] * p[g + "_gss"]
                nc.vector.tensor_scalar(S, S,
                                        float(1.0 / (p[g + "_p_sat"] * ags)),
                                        float(1.0 / ags),
                                        op0=OP.mult, op1=OP.add)
                Sm[(rb, g)] = S
            nc.vector.reciprocal(Sall[:], Sall[:])

        # Phase B: v = (x+bias)*gain in place, tanh, amax, store
        for rb in range(2):
            t = xt[rb]
            x3 = t[:].rearrange("p (w c) -> p w c", c=C)
            sp = x3[:, :, 0:3]
            nc.vector.tensor_scalar(
                sp, sp, float(p["special_alpha"]),
                float(p["special_alpha"] * p["special_bias"]),
                op0=OP.mult, op1=OP.add)
            for g, c0, c1 in GROUPS:
                cg = c1 - c0
                gb = Sm[(rb, g)].unsqueeze(2).to_broadcast([128, W, cg])
                nc.vector.scalar_tensor_tensor(
                    x3[:, :, c0:c1], x3[:, :, c0:c1],
                    float(p[g + "_bias"]), gb, op0=OP.add, op1=OP.mult)
            nc.scalar.activation(t[:], t[:], AF.Tanh)
            # final amax scale: GPSIMD is pathologically slow on strided APs,
            # so split between DVE (special+low+mid) and ACT (high).
            nc.vector.tensor_scalar_mul(x3[:, :, 0:3], x3[:, :, 0:3],
                                        float(p["special_amax"]))
            nc.vector.tensor_scalar_mul(x3[:, :, 3:6], x3[:, :, 3:6],
                                        float(p["low_amax"]))
            nc.vector.tensor_scalar_mul(x3[:, :, 6:15], x3[:, :, 6:15],
                                        float(p["mid_amax"]))
            nc.scalar.mul(x3[:, :, 15:36], x3[:, :, 15:36],
                          float(p["high_amax"]))
            nc.sync.dma_start(y[rb * 128 : (rb + 1) * 128, :], t[:])

    nc.compile()
    return nc


_SCALARS = [
    "special_bias", "special_alpha", "special_amax", "special_eps",
    "low_bias", "low_alpha", "low_amax", "low_eps", "low_gss", "low_p_sat",
    "mid_bias", "mid_alpha", "mid_amax", "mid_eps", "mid_gss", "mid_p_sat",
    "high_bias", "high_alpha", "high_amax", "high_eps", "high_gss",
    "high_p_sat",
]


def build_nc(**inputs):
    """Build (or fetch cached) compiled Bass program for these scalar params."""
    p = {k: float(np.asarray(inputs[k]).reshape(-1)[0]) for k in _SCALARS}
    key = tuple(p[k] for k in _SCALARS)
    if key not in _NC_CACHE:
        _NC_CACHE[key] = _build(p)
    return _NC_CACHE[key]


def kernel(**inputs) -> np.ndarray:
    from concourse.bass_utils import run_bass_kernel_spmd

    raw = np.ascontiguousarray(np.asarray(inputs["raw_coeffs"],
                                          dtype=np.float32))
    assert raw.shape == (B, H, W, C), raw.shape
    nc = build_nc(**inputs)
    in_maps = [{"x": raw[i].reshape(H, FD)} for i in range(N_CORES)]
    res = run_bass_kernel_spmd(nc, in_maps, list(range(N_CORES)))
    out = np.stack([res.results[i]["y"].reshape(H, W, C)
                    for i in range(N_CORES)])
    return out.astype(np.float32)


# revision 10
# speedup vs baseline: 2.7086x; 1.1524x over previous
"""Trainium2 Bass kernel for nn_LocalGroupedZernikeNewBP.

Full inputs in, full output out. Shards raw_coeffs [8,256,256,36] along the
batch dim: one image per NeuronCore (8 cores). Scalar params are baked into
the compiled program as immediates (rebuilt per distinct param values).

Per-core program (image [H=256, W=256, C=36], HWC contiguous):
  - 2 row-blocks of 128 rows x 2 column-halves of 128 cols (4 work units,
    SBUF tiles [128, 128*36]; DMAs contiguous 18KB rows).
  - special (ch 0:3):  out = amax * tanh(alpha*(x+bias))
  - joint groups low(3:6) mid(6:15) high(15:36):
      soft_abs = sqrt((x+bias)^2 + eps)     (GPSIMD square when bias==0,
                                             else ACT Square; ACT Sqrt)
      chansum  = sum_c soft_abs             (DVE reduce over C)
      s        = 3x3 box(chansum), edge-replicated (DVE shifted adds; row
                 shifts via SBUF->SBUF DMA, halo between the row-blocks)
      G        = alpha*gss/(1 + s/p_sat)   (DVE, one reciprocal per block)
      v        = (x+bias) * G               (DVE scalar_tensor_tensor,
                                             G broadcast over C)
      out      = amax * tanh(v)             (ACT Tanh whole tile; amax via
                                             DVE ch 0:15 + ACT ch 15:36)
"""

import numpy as np

B, H, W, C = 8, 256, 256, 36
WH = W // 2          # column-half width
FD = W * C           # free elements per full row
FDH = WH * C         # free elements per half row
GROUPS = [("low", 3, 6), ("mid", 6, 15), ("high", 15, 36)]
N_CORES = 8

_NC_CACHE: dict[tuple, object] = {}


def _build(p: dict[str, float]):
    from contextlib import ExitStack

    import concourse.bass as bass  # noqa: F401
    import concourse.tile as tile
    from concourse import bacc, mybir

    f32 = mybir.dt.float32
    AF = mybir.ActivationFunctionType
    OP = mybir.AluOpType
    AX = mybir.AxisListType

    nc = bacc.Bacc(
        "TRN2", target_bir_lowering=False, debug=False, num_devices=N_CORES
    )
    x = nc.dram_tensor("x", [H, FD], f32, kind="ExternalInput").ap()
    y = nc.dram_tensor("y", [H, FD], f32, kind="ExternalOutput").ap()

    joint_bias_zero = all(p[g + "_bias"] == 0.0 for g, _, _ in GROUPS)

    with tile.TileContext(nc) as tc, ExitStack() as ctx:
        xp = ctx.enter_context(tc.tile_pool(name="xp", bufs=4))
        sqp = ctx.enter_context(tc.tile_pool(name="sqp", bufs=3))
        mp = ctx.enter_context(tc.tile_pool(name="maps", bufs=1))
        cp = ctx.enter_context(tc.tile_pool(name="consts", bufs=1))

        _consts: dict[float, object] = {}

        def constant(val: float):
            """[128,1] SBUF tile holding `val` (for ACT bias operands)."""
            val = float(val)
            if val not in _consts:
                ct = cp.tile([128, 1], f32, tag=f"const{len(_consts)}")
                nc.vector.memset(ct[:], val)
                _consts[val] = ct
            return _consts[val][:]

        xt = {}
        cs = {}
        Tm = {}
        Sall = {}

        # Phase A: load, soft_abs, channel sums (per row-block x col-half)
        for rb in range(2):
            for g, _, _ in GROUPS:
                cs[(rb, g)] = mp.tile([128, W], f32, tag=f"cs{rb}{g}",
                                      name=f"cs{rb}{g}")
            for h in range(2):
                t = xp.tile([128, FDH], f32, tag="x")
                nc.sync.dma_start(
                    t[:], x[rb * 128 : (rb + 1) * 128,
                            h * FDH : (h + 1) * FDH])
                xt[(rb, h)] = t
                x3 = t[:].rearrange("p (w c) -> p w c", c=C)
                sq = sqp.tile([128, FDH], f32, tag="sq")
                sq3 = sq[:].rearrange("p (w c) -> p w c", c=C)
                if joint_bias_zero:
                    nc.gpsimd.tensor_tensor(sq[:], t[:], t[:], op=OP.mult)
                else:
                    for g, c0, c1 in GROUPS:
                        nc.scalar.activation(sq3[:, :, c0:c1],
                                             x3[:, :, c0:c1], AF.Square,
                                             bias=constant(p[g + "_bias"]))
                for g, c0, c1 in GROUPS:
                    nc.scalar.activation(sq3[:, :, c0:c1], sq3[:, :, c0:c1],
                                         AF.Sqrt,
                                         bias=constant(p[g + "_eps"]))
                for g, c0, c1 in GROUPS:
                    nc.vector.reduce_sum(
                        cs[(rb, g)][:, h * WH : (h + 1) * WH],
                        sq3[:, :, c0:c1], axis=AX.X)

            # W-direction 3-tap box with replicate edges (full width)
            for g, c0, c1 in GROUPS:
                c_ = cs[(rb, g)]
                T = mp.tile([128, W], f32, tag=f"T{rb}{g}")
                nc.vector.tensor_add(T[:, 1 : W - 1], c_[:, 0 : W - 2],
                                     c_[:, 2:W])
                nc.vector.tensor_add(T[:, 0:1], c_[:, 0:1], c_[:, 1:2])
                nc.vector.tensor_add(T[:, W - 1 : W], c_[:, W - 2 : W - 1],
                                     c_[:, W - 1 : W])
                nc.vector.tensor_add(T[:], T[:], c_[:])
                Tm[(rb, g)] = T

        # Phase A2: H-direction 3-tap box (halo row from the other block).
        # Compute-engine SBUF APs must start at partition 0/32/64/96, so the
        # +-1 row shifts use SBUF->SBUF DMA (SWDGE via gpsimd),
        # then aligned adds. The 3 group maps of a block share one
        # tile so one reciprocal per block gives
        # G = 1/(s/(p_sat*alpha*gss) + 1/(alpha*gss)) = alpha*gss/(1+s/p_sat).
        for rb in range(2):
            Sa = mp.tile([128, 3 * W], f32, tag=f"Sall{rb}")
            Sall[rb] = Sa
            for gi, (g, c0, c1) in enumerate(GROUPS):
                T = Tm[(rb, g)]
                To = Tm[(1 - rb, g)]
                U = mp.tile([128, W], f32, tag=f"U{rb}{g}")
                D = mp.tile([128, W], f32, tag=f"D{rb}{g}")
                nc.gpsimd.dma_start(U[1:128, :], T[0:127, :])
                nc.gpsimd.dma_start(D[0:127, :], T[1:128, :])
                if rb == 0:
                    nc.gpsimd.dma_start(U[0:1, :], T[0:1, :])
                    nc.gpsimd.dma_start(D[127:128, :], To[0:1, :])
                else:
                    nc.gpsimd.dma_start(U[0:1, :], To[127:128, :])
                    nc.gpsimd.dma_start(D[127:128, :], T[127:128, :])
                S = Sa[:, gi * W : (gi + 1) * W]
                nc.vector.tensor_add(S, T[:], U[:])
                nc.vector.tensor_add(S, S, D[:])
                ags = p[g + "_alpha"] * p[g + "_gss"]
                nc.vector.tensor_scalar(S, S,
                                        float(1.0 / (p[g + "_p_sat"] * ags)),
                                        float(1.0 / ags),
                                        op0=OP.mult, op1=OP.add)
            nc.vector.reciprocal(Sa[:], Sa[:])

        # Phase B: v = (x+bias)/r in place, tanh, amax, store
        for rb in range(2):
            for h in range(2):
                t = xt[(rb, h)]
                x3 = t[:].rearrange("p (w c) -> p w c", c=C)
                sp = x3[:, :, 0:3]
                nc.vector.tensor_scalar(
                    sp, sp, float(p["special_alpha"]),
                    float(p["special_alpha"] * p["special_bias"]),
                    op0=OP.mult, op1=OP.add)
                for gi, (g, c0, c1) in enumerate(GROUPS):
                    cg = c1 - c0
                    rsl = Sall[rb][:, gi * W + h * WH : gi * W + (h + 1) * WH]
                    rb_ = rsl.unsqueeze(2).to_broadcast([128, WH, cg])
                    nc.vector.scalar_tensor_tensor(
                        x3[:, :, c0:c1], x3[:, :, c0:c1],
                        float(p[g + "_bias"]), rb_,
                        op0=OP.add, op1=OP.mult)
                nc.scalar.activation(t[:], t[:], AF.Tanh)
                # final amax scale: DVE ch 0:15, ACT ch 15:36
                nc.vector.tensor_scalar_mul(x3[:, :, 0:3], x3[:, :, 0:3],
                                            float(p["special_amax"]))
                nc.vector.tensor_scalar_mul(x3[:, :, 3:6], x3[:, :, 3:6],
                                            float(p["low_amax"]))
                nc.vector.tensor_scalar_mul(x3[:, :, 6:15], x3[:, :, 6:15],
                                            float(p["mid_amax"]))
                nc.scalar.mul(x3[:, :, 15:36], x3[:, :, 15:36],
                              float(p["high_amax"]))
                nc.sync.dma_start(
                    y[rb * 128 : (rb + 1) * 128, h * FDH : (h + 1) * FDH],
                    t[:])

    nc.compile()
    return nc


_SCALARS = [
    "special_bias", "special_alpha", "special_amax", "special_eps",
    "low_bias", "low_alpha", "low_amax", "low_eps", "low_gss", "low_p_sat",
    "mid_bias", "mid_alpha", "mid_amax", "mid_eps", "mid_gss", "mid_p_sat",
    "high_bias", "high_alpha", "high_amax", "high_eps", "high_gss",
    "high_p_sat",
]


def build_nc(**inputs):
    """Build (or fetch cached) compiled Bass program for these scalar params."""
    p = {k: float(np.asarray(inputs[k]).reshape(-1)[0]) for k in _SCALARS}
    key = tuple(p[k] for k in _SCALARS)
    if key not in _NC_CACHE:
        _NC_CACHE[key] = _build(p)
    return _NC_CACHE[key]


def kernel(**inputs) -> np.ndarray:
    from concourse.bass_utils import run_bass_kernel_spmd

    raw = np.ascontiguousarray(np.asarray(inputs["raw_coeffs"],
                                          dtype=np.float32))
    assert raw.shape == (B, H, W, C), raw.shape
    nc = build_nc(**inputs)
    in_maps = [{"x": raw[i].reshape(H, FD)} for i in range(N_CORES)]
    res = run_bass_kernel_spmd(nc, in_maps, list(range(N_CORES)))
    out = np.stack([res.results[i]["y"].reshape(H, W, C)
                    for i in range(N_CORES)])
    return out.astype(np.float32)


# revision 13
# speedup vs baseline: 2.9426x; 1.0864x over previous
"""Trainium2 Bass kernel for nn_LocalGroupedZernikeNewBP.

Full inputs in, full output out. Shards raw_coeffs [8,256,256,36] along the
batch dim: one image per NeuronCore (8 cores). Scalar params are baked into
the compiled program as immediates (rebuilt per distinct param values).

Per-core program (image [H=256, W=256, C=36], HWC contiguous):
  - 2 row-blocks of 128 rows x 2 column-halves of 128 cols (4 work units,
    SBUF tiles [128, 128*36]; DMAs contiguous 18KB rows).
  - special (ch 0:3):  out = amax * tanh(alpha*(x+bias))
  - joint groups low(3:6) mid(6:15) high(15:36):
      soft_abs = sqrt((x+bias)^2 + eps)     (GPSIMD square when bias==0,
                                             else ACT Square; ACT Sqrt)
      chansum  = sum_c soft_abs             (DVE reduce over C)
      s        = 3x3 box(chansum), edge-replicated (DVE shifted adds; row
                 shifts via SBUF->SBUF DMA, halo between the row-blocks)
      G        = alpha*gss/(1 + s/p_sat)   (DVE, one reciprocal per block)
      v        = (x+bias) * G               (DVE scalar_tensor_tensor,
                                             G broadcast over C)
      out      = amax * tanh(v)             (ACT Tanh whole tile; amax via
                                             DVE ch 0:15 + ACT ch 15:36)
"""

import numpy as np

B, H, W, C = 8, 256, 256, 36
WH = W // 2          # column-half width
FD = W * C           # free elements per full row
FDH = WH * C         # free elements per half row
GROUPS = [("low", 3, 6), ("mid", 6, 15), ("high", 15, 36)]
N_CORES = 8

_NC_CACHE: dict[tuple, object] = {}


def _build(p: dict[str, float]):
    from contextlib import ExitStack

    import concourse.bass as bass  # noqa: F401
    import concourse.tile as tile
    from concourse import bacc, mybir

    f32 = mybir.dt.float32
    AF = mybir.ActivationFunctionType
    OP = mybir.AluOpType
    AX = mybir.AxisListType

    nc = bacc.Bacc(
        "TRN2", target_bir_lowering=False, debug=False, num_devices=N_CORES
    )
    x = nc.dram_tensor("x", [H, FD], f32, kind="ExternalInput").ap()
    bands = nc.dram_tensor("bands", [128, 256], f32,
                           kind="ExternalInput").ap()
    halos = nc.dram_tensor("halos", [1, 256], f32, kind="ExternalInput").ap()
    y = nc.dram_tensor("y", [H, FD], f32, kind="ExternalOutput").ap()

    joint_bias_zero = all(p[g + "_bias"] == 0.0 for g, _, _ in GROUPS)

    with tile.TileContext(nc) as tc, ExitStack() as ctx:
        xp = ctx.enter_context(tc.tile_pool(name="xp", bufs=4))
        sqp = ctx.enter_context(tc.tile_pool(name="sqp", bufs=3))
        mp = ctx.enter_context(tc.tile_pool(name="maps", bufs=1))
        cp = ctx.enter_context(tc.tile_pool(name="consts", bufs=1))
        psp = ctx.enter_context(tc.tile_pool(name="psum", bufs=3,
                                             space="PSUM"))

        _consts: dict[float, object] = {}

        def constant(val: float):
            """[128,1] SBUF tile holding `val` (for ACT bias operands)."""
            val = float(val)
            if val not in _consts:
                ct = cp.tile([128, 1], f32, tag=f"const{len(_consts)}")
                nc.vector.memset(ct[:], val)
                _consts[val] = ct
            return _consts[val][:]

        xt = {}
        cs = {}
        Tm = {}
        Sall = {}

        # Phase A: load, soft_abs, channel sums (per row-block x col-half)
        for rb in range(2):
            for g, _, _ in GROUPS:
                cs[(rb, g)] = mp.tile([128, W], f32, tag=f"cs{rb}{g}",
                                      name=f"cs{rb}{g}")
            for h in range(2):
                t = xp.tile([128, FDH], f32, tag="x")
                nc.sync.dma_start(
                    t[:], x[rb * 128 : (rb + 1) * 128,
                            h * FDH : (h + 1) * FDH])
                xt[(rb, h)] = t
                x3 = t[:].rearrange("p (w c) -> p w c", c=C)
                sq = sqp.tile([128, FDH], f32, tag="sq")
                sq3 = sq[:].rearrange("p (w c) -> p w c", c=C)
                if joint_bias_zero:
                    nc.gpsimd.tensor_tensor(sq[:], t[:], t[:], op=OP.mult)
                else:
                    for g, c0, c1 in GROUPS:
                        nc.scalar.activation(sq3[:, :, c0:c1],
                                             x3[:, :, c0:c1], AF.Square,
                                             bias=constant(p[g + "_bias"]))
                for g, c0, c1 in GROUPS:
                    nc.scalar.activation(sq3[:, :, c0:c1], sq3[:, :, c0:c1],
                                         AF.Sqrt,
                                         bias=constant(p[g + "_eps"]))
                for g, c0, c1 in GROUPS:
                    nc.vector.reduce_sum(
                        cs[(rb, g)][:, h * WH : (h + 1) * WH],
                        sq3[:, :, c0:c1], axis=AX.X)

            # W-direction 3-tap box with replicate edges (full width)
            for g, c0, c1 in GROUPS:
                c_ = cs[(rb, g)]
                T = mp.tile([128, W], f32, tag=f"T{rb}{g}")
                nc.vector.tensor_add(T[:, 1 : W - 1], c_[:, 0 : W - 2],
                                     c_[:, 2:W])
                nc.vector.tensor_add(T[:, 0:1], c_[:, 0:1], c_[:, 1:2])
                nc.vector.tensor_add(T[:, W - 1 : W], c_[:, W - 2 : W - 1],
                                     c_[:, W - 1 : W])
                nc.vector.tensor_add(T[:], T[:], c_[:])
                Tm[(rb, g)] = T

        # Phase A2: H-direction 3-tap box on TensorE: S = band.T @ T plus a
        # rank-1 halo matmul for the row from the other block (replicate
        # edges are baked into the band matrices, passed in as inputs).
        # PE rhs partition APs must start at 0/32/64/96, so block 1's halo
        # row (T0 row 127) is first DMA-copied to partition 0 of a scratch
        # tile. The 3 group maps of a block share one SBUF tile so one
        # reciprocal per block gives
        # G = 1/(s/(p_sat*alpha*gss) + 1/(alpha*gss)) = alpha*gss/(1+s/p_sat).
        bands_t = cp.tile([128, 256], f32, tag="bands")
        halos_t = cp.tile([1, 256], f32, tag="halos")
        nc.sync.dma_start(bands_t[:], bands[:])
        nc.sync.dma_start(halos_t[:], halos[:])
        hrow = {}
        for g, c0, c1 in GROUPS:
            hr = mp.tile([1, W], f32, tag=f"hrow{g}", name=f"hrow{g}")
            nc.gpsimd.dma_start(hr[:], Tm[(0, g)][127:128, :])
            hrow[g] = hr
        for rb in range(2):
            Sa = mp.tile([128, 3 * W], f32, tag=f"Sall{rb}")
            Sall[rb] = Sa
            for gi, (g, c0, c1) in enumerate(GROUPS):
                S_ps = psp.tile([128, W], f32, tag="ps")
                nc.tensor.matmul(S_ps[:],
                                 bands_t[:, rb * 128 : (rb + 1) * 128],
                                 Tm[(rb, g)][:], start=True, stop=False)
                halo_rhs = Tm[(1, g)][0:1, :] if rb == 0 else hrow[g][:]
                nc.tensor.matmul(S_ps[:],
                                 halos_t[0:1, rb * 128 : (rb + 1) * 128],
                                 halo_rhs, start=False, stop=True)
                S = Sa[:, gi * W : (gi + 1) * W]
                ags = p[g + "_alpha"] * p[g + "_gss"]
                nc.vector.tensor_scalar(S, S_ps[:],
                                        float(1.0 / (p[g + "_p_sat"] * ags)),
                                        float(1.0 / ags),
                                        op0=OP.mult, op1=OP.add)
            nc.vector.reciprocal(Sa[:], Sa[:])

        # Phase B: v = (x+bias)/r in place, tanh, amax, store
        for rb in range(2):
            for h in range(2):
                t = xt[(rb, h)]
                x3 = t[:].rearrange("p (w c) -> p w c", c=C)
                sp = x3[:, :, 0:3]
                nc.scalar.activation(
                    sp, sp, AF.Copy,
                    bias=float(p["special_alpha"] * p["special_bias"]),
                    scale=float(p["special_alpha"]))
                for gi, (g, c0, c1) in enumerate(GROUPS):
                    cg = c1 - c0
                    rsl = Sall[rb][:, gi * W + h * WH : gi * W + (h + 1) * WH]
                    rb_ = rsl.unsqueeze(2).to_broadcast([128, WH, cg])
                    nc.vector.scalar_tensor_tensor(
                        x3[:, :, c0:c1], x3[:, :, c0:c1],
                        float(p[g + "_bias"]), rb_,
                        op0=OP.add, op1=OP.mult)
                nc.scalar.activation(t[:], t[:], AF.Tanh)
                # final amax scale: DVE ch 0:6, ACT ch 6:36
                nc.vector.tensor_scalar_mul(x3[:, :, 0:3], x3[:, :, 0:3],
                                            float(p["special_amax"]))
                nc.vector.tensor_scalar_mul(x3[:, :, 3:6], x3[:, :, 3:6],
                                            float(p["low_amax"]))
                nc.scalar.mul(x3[:, :, 6:15], x3[:, :, 6:15],
                              float(p["mid_amax"]))
                nc.scalar.mul(x3[:, :, 15:36], x3[:, :, 15:36],
                              float(p["high_amax"]))
                nc.sync.dma_start(
                    y[rb * 128 : (rb + 1) * 128, h * FDH : (h + 1) * FDH],
                    t[:])

    nc.compile()
    return nc


_SCALARS = [
    "special_bias", "special_alpha", "special_amax", "special_eps",
    "low_bias", "low_alpha", "low_amax", "low_eps", "low_gss", "low_p_sat",
    "mid_bias", "mid_alpha", "mid_amax", "mid_eps", "mid_gss", "mid_p_sat",
    "high_bias", "high_alpha", "high_amax", "high_eps", "high_gss",
    "high_p_sat",
]


def build_nc(**inputs):
    """Build (or fetch cached) compiled Bass program for these scalar params."""
    p = {k: float(np.asarray(inputs[k]).reshape(-1)[0]) for k in _SCALARS}
    key = tuple(p[k] for k in _SCALARS)
    if key not in _NC_CACHE:
        _NC_CACHE[key] = _build(p)
    return _NC_CACHE[key]


def _band_arrays():
    """Band matrices (lhsT, [k, m] = contribution of input row k to output
    row m) for the H-direction 3-tap box, replicate edges baked in, plus
    rank-1 halo row selectors."""
    A = np.zeros((128, 128), np.float32)
    for m in range(128):
        for k in (m - 1, m, m + 1):
            if 0 <= k < 128:
                A[k, m] = 1.0
    A0 = A.copy()
    A0[0, 0] = 2.0       # top replicate (block 0)
    A1 = A.copy()
    A1[127, 127] = 2.0   # bottom replicate (block 1)
    bands = np.concatenate([A0, A1], axis=1)          # [128, 256]
    halos = np.zeros((1, 256), np.float32)
    halos[0, 127] = 1.0      # block 0: out[127] += T1[0]
    halos[0, 128 + 0] = 1.0  # block 1: out[0]   += T0[127]
    return bands, halos


def kernel(**inputs) -> np.ndarray:
    from concourse.bass_utils import run_bass_kernel_spmd

    raw = np.ascontiguousarray(np.asarray(inputs["raw_coeffs"],
                                          dtype=np.float32))
    assert raw.shape == (B, H, W, C), raw.shape
    nc = build_nc(**inputs)
    bands, halos = _band_arrays()
    in_maps = [{"x": raw[i].reshape(H, FD), "bands": bands, "halos": halos}
               for i in range(N_CORES)]
    res = run_bass_kernel_spmd(nc, in_maps, list(range(N_CORES)))
    out = np.stack([res.results[i]["y"].reshape(H, W, C)
                    for i in range(N_CORES)])
    return out.astype(np.float32)


# revision 14
# speedup vs baseline: 3.8509x; 1.3087x over previous
"""Trainium2 Bass kernel for nn_LocalGroupedZernikeNewBP.

Full inputs in, full output out. Shards raw_coeffs [8,256,256,36] along the
batch dim: one image per NeuronCore (8 cores). Scalar params are baked into
the compiled program as immediates (rebuilt per distinct param values).

Per-core program (image [H=256, W=256, C=36], HWC contiguous):
  - 2 row-blocks of 128 rows x 2 column-halves of 128 cols (4 work units,
    SBUF tiles [128, 128*36]; DMAs contiguous 18KB rows).
  - special (ch 0:3):  out = amax * tanh(alpha*(x+bias))
  - joint groups low(3:6) mid(6:15) high(15:36):
      soft_abs = sqrt((x+bias)^2 + eps)     (GPSIMD square when bias==0,
                                             else ACT Square; ACT Sqrt)
      chansum  = sum_c soft_abs             (DVE reduce over C)
      s        = 3x3 box(chansum), edge-replicated (DVE shifted adds; row
                 shifts via SBUF->SBUF DMA, halo between the row-blocks)
      G        = alpha*gss/(1 + s/p_sat)   (DVE, one reciprocal per block)
      v        = (x+bias) * G               (DVE scalar_tensor_tensor,
                                             G broadcast over C)
      out      = amax * tanh(v)             (ACT Tanh whole tile; amax via
                                             DVE ch 0:15 + ACT ch 15:36)
"""

import numpy as np

B, H, W, C = 8, 256, 256, 36
WH = W // 2          # column-half width
FD = W * C           # free elements per full row
FDH = WH * C         # free elements per half row
GROUPS = [("low", 3, 6), ("mid", 6, 15), ("high", 15, 36)]
N_CORES = 8

_NC_CACHE: dict[tuple, object] = {}


def _build(p: dict[str, float]):
    from contextlib import ExitStack

    import concourse.bass as bass  # noqa: F401
    import concourse.tile as tile
    from concourse import bacc, mybir

    f32 = mybir.dt.float32
    AF = mybir.ActivationFunctionType
    OP = mybir.AluOpType
    AX = mybir.AxisListType

    nc = bacc.Bacc(
        "TRN2", target_bir_lowering=False, debug=False, num_devices=N_CORES
    )
    x = nc.dram_tensor("x", [H, FD], f32, kind="ExternalInput").ap()
    bands = nc.dram_tensor("bands", [128, 256], f32,
                           kind="ExternalInput").ap()
    halos = nc.dram_tensor("halos", [1, 256], f32, kind="ExternalInput").ap()
    y = nc.dram_tensor("y", [H, FD], f32, kind="ExternalOutput").ap()

    joint_bias_zero = all(p[g + "_bias"] == 0.0 for g, _, _ in GROUPS)

    with tile.TileContext(nc) as tc, ExitStack() as ctx:
        xp = ctx.enter_context(tc.tile_pool(name="xp", bufs=4))
        sqp = ctx.enter_context(tc.tile_pool(name="sqp", bufs=3))
        mp = ctx.enter_context(tc.tile_pool(name="maps", bufs=1))
        cp = ctx.enter_context(tc.tile_pool(name="consts", bufs=1))
        psp = ctx.enter_context(tc.tile_pool(name="psum", bufs=3,
                                             space="PSUM"))

        _consts: dict[float, object] = {}

        def constant(val: float):
            """[128,1] SBUF tile holding `val` (for ACT bias operands)."""
            val = float(val)
            if val not in _consts:
                ct = cp.tile([128, 1], f32, tag=f"const{len(_consts)}")
                nc.vector.memset(ct[:], val)
                _consts[val] = ct
            return _consts[val][:]

        xt = {}
        cs = {}
        Tm = {}
        Sall = {}

        # Phase A: load, soft_abs, channel sums (per row-block x col-half)
        for rb in range(2):
            for g, _, _ in GROUPS:
                cs[(rb, g)] = mp.tile([128, W], f32, tag=f"cs{rb}{g}",
                                      name=f"cs{rb}{g}")
            for h in range(2):
                t = xp.tile([128, FDH], f32, tag="x")
                nc.sync.dma_start(
                    t[:], x[rb * 128 : (rb + 1) * 128,
                            h * FDH : (h + 1) * FDH])
                xt[(rb, h)] = t
                x3 = t[:].rearrange("p (w c) -> p w c", c=C)
                sq = sqp.tile([128, FDH], f32, tag="sq")
                sq3 = sq[:].rearrange("p (w c) -> p w c", c=C)
                if joint_bias_zero:
                    # one contiguous Square over ch 3:36 (GPSIMD would be
                    # faster in isolation but steals DVE SBUF ports)
                    nc.scalar.activation(
                        sq[:].rearrange("p (w c) -> p w c", c=C)[:, :, 3:36],
                        x3[:, :, 3:36], AF.Square)
                else:
                    for g, c0, c1 in GROUPS:
                        nc.scalar.activation(sq3[:, :, c0:c1],
                                             x3[:, :, c0:c1], AF.Square,
                                             bias=constant(p[g + "_bias"]))
                for g, c0, c1 in GROUPS:
                    nc.scalar.activation(sq3[:, :, c0:c1], sq3[:, :, c0:c1],
                                         AF.Sqrt,
                                         bias=constant(p[g + "_eps"]))
                for g, c0, c1 in GROUPS:
                    nc.vector.reduce_sum(
                        cs[(rb, g)][:, h * WH : (h + 1) * WH],
                        sq3[:, :, c0:c1], axis=AX.X)

            # W-direction 3-tap box with replicate edges (full width)
            for g, c0, c1 in GROUPS:
                c_ = cs[(rb, g)]
                T = mp.tile([128, W], f32, tag=f"T{rb}{g}")
                nc.vector.tensor_add(T[:, 1 : W - 1], c_[:, 0 : W - 2],
                                     c_[:, 2:W])
                nc.vector.tensor_add(T[:, 0:1], c_[:, 0:1], c_[:, 1:2])
                nc.vector.tensor_add(T[:, W - 1 : W], c_[:, W - 2 : W - 1],
                                     c_[:, W - 1 : W])
                nc.vector.tensor_add(T[:], T[:], c_[:])
                Tm[(rb, g)] = T

        # Phase A2: H-direction 3-tap box on TensorE: S = band.T @ T plus a
        # rank-1 halo matmul for the row from the other block (replicate
        # edges are baked into the band matrices, passed in as inputs).
        # PE rhs partition APs must start at 0/32/64/96, so block 1's halo
        # row (T0 row 127) is first DMA-copied to partition 0 of a scratch
        # tile. The 3 group maps of a block share one SBUF tile so one
        # reciprocal per block gives
        # G = 1/(s/(p_sat*alpha*gss) + 1/(alpha*gss)) = alpha*gss/(1+s/p_sat).
        bands_t = cp.tile([128, 256], f32, tag="bands")
        halos_t = cp.tile([1, 256], f32, tag="halos")
        nc.sync.dma_start(bands_t[:], bands[:])
        nc.sync.dma_start(halos_t[:], halos[:])
        hrow = {}
        for g, c0, c1 in GROUPS:
            hr = mp.tile([1, W], f32, tag=f"hrow{g}", name=f"hrow{g}")
            nc.gpsimd.dma_start(hr[:], Tm[(0, g)][127:128, :])
            hrow[g] = hr
        for rb in range(2):
            Sa = mp.tile([128, 3 * W], f32, tag=f"Sall{rb}")
            Sall[rb] = Sa
            for gi, (g, c0, c1) in enumerate(GROUPS):
                S_ps = psp.tile([128, W], f32, tag="ps")
                nc.tensor.matmul(S_ps[:],
                                 bands_t[:, rb * 128 : (rb + 1) * 128],
                                 Tm[(rb, g)][:], start=True, stop=False)
                halo_rhs = Tm[(1, g)][0:1, :] if rb == 0 else hrow[g][:]
                nc.tensor.matmul(S_ps[:],
                                 halos_t[0:1, rb * 128 : (rb + 1) * 128],
                                 halo_rhs, start=False, stop=True)
                S = Sa[:, gi * W : (gi + 1) * W]
                ags = p[g + "_alpha"] * p[g + "_gss"]
                nc.vector.tensor_scalar(S, S_ps[:],
                                        float(1.0 / (p[g + "_p_sat"] * ags)),
                                        float(1.0 / ags),
                                        op0=OP.mult, op1=OP.add)
            nc.vector.reciprocal(Sa[:], Sa[:])

        # Phase B: v = (x+bias)/r in place, tanh, amax, store
        for rb in range(2):
            for h in range(2):
                t = xt[(rb, h)]
                x3 = t[:].rearrange("p (w c) -> p w c", c=C)
                sp = x3[:, :, 0:3]
                if p["special_alpha"] != 1.0 or p["special_bias"] != 0.0:
                    nc.scalar.activation(
                        sp, sp, AF.Copy,
                        bias=float(p["special_alpha"] * p["special_bias"]),
                        scale=float(p["special_alpha"]))
                for gi, (g, c0, c1) in enumerate(GROUPS):
                    cg = c1 - c0
                    rsl = Sall[rb][:, gi * W + h * WH : gi * W + (h + 1) * WH]
                    rb_ = rsl.unsqueeze(2).to_broadcast([128, WH, cg])
                    nc.vector.scalar_tensor_tensor(
                        x3[:, :, c0:c1], x3[:, :, c0:c1],
                        float(p[g + "_bias"]), rb_,
                        op0=OP.add, op1=OP.mult)
                nc.scalar.activation(t[:], t[:], AF.Tanh)
                # final amax scale (skipped when amax == 1):
                # DVE ch 0:6, ACT ch 6:36
                if p["special_amax"] != 1.0:
                    nc.vector.tensor_scalar_mul(x3[:, :, 0:3], x3[:, :, 0:3],
                                                float(p["special_amax"]))
                if p["low_amax"] != 1.0:
                    nc.vector.tensor_scalar_mul(x3[:, :, 3:6], x3[:, :, 3:6],
                                                float(p["low_amax"]))
                if p["mid_amax"] != 1.0:
                    nc.scalar.mul(x3[:, :, 6:15], x3[:, :, 6:15],
                                  float(p["mid_amax"]))
                if p["high_amax"] != 1.0:
                    nc.scalar.mul(x3[:, :, 15:36], x3[:, :, 15:36],
                                  float(p["high_amax"]))
                nc.sync.dma_start(
                    y[rb * 128 : (rb + 1) * 128, h * FDH : (h + 1) * FDH],
                    t[:])

    nc.compile()
    return nc


_SCALARS = [
    "special_bias", "special_alpha", "special_amax", "special_eps",
    "low_bias", "low_alpha", "low_amax", "low_eps", "low_gss", "low_p_sat",
    "mid_bias", "mid_alpha", "mid_amax", "mid_eps", "mid_gss", "mid_p_sat",
    "high_bias", "high_alpha", "high_amax", "high_eps", "high_gss",
    "high_p_sat",
]


def build_nc(**inputs):
    """Build (or fetch cached) compiled Bass program for these scalar params."""
    p = {k: float(np.asarray(inputs[k]).reshape(-1)[0]) for k in _SCALARS}
    key = tuple(p[k] for k in _SCALARS)
    if key not in _NC_CACHE:
        _NC_CACHE[key] = _build(p)
    return _NC_CACHE[key]


def _band_arrays():
    """Band matrices (lhsT, [k, m] = contribution of input row k to output
    row m) for the H-direction 3-tap box, replicate edges baked in, plus
    rank-1 halo row selectors."""
    A = np.zeros((128, 128), np.float32)
    for m in range(128):
        for k in (m - 1, m, m + 1):
            if 0 <= k < 128:
                A[k, m] = 1.0
    A0 = A.copy()
    A0[0, 0] = 2.0       # top replicate (block 0)
    A1 = A.copy()
    A1[127, 127] = 2.0   # bottom replicate (block 1)
    bands = np.concatenate([A0, A1], axis=1)          # [128, 256]
    halos = np.zeros((1, 256), np.float32)
    halos[0, 127] = 1.0      # block 0: out[127] += T1[0]
    halos[0, 128 + 0] = 1.0  # block 1: out[0]   += T0[127]
    return bands, halos


def kernel(**inputs) -> np.ndarray:
    from concourse.bass_utils import run_bass_kernel_spmd

    raw = np.ascontiguousarray(np.asarray(inputs["raw_coeffs"],
                                          dtype=np.float32))
    assert raw.shape == (B, H, W, C), raw.shape
    nc = build_nc(**inputs)
    bands, halos = _band_arrays()
    in_maps = [{"x": raw[i].reshape(H, FD), "bands": bands, "halos": halos}
               for i in range(N_CORES)]
    res = run_bass_kernel_spmd(nc, in_maps, list(range(N_CORES)))
    out = np.stack([res.results[i]["y"].reshape(H, W, C)
                    for i in range(N_CORES)])
    return out.astype(np.float32)


# revision 15
# speedup vs baseline: 4.0735x; 1.0578x over previous
"""Trainium2 Bass kernel for nn_LocalGroupedZernikeNewBP.

Full inputs in, full output out. Shards raw_coeffs [8,256,256,36] along the
batch dim: one image per NeuronCore (8 cores). Scalar params are baked into
the compiled program as immediates (rebuilt per distinct param values).

Per-core program (image [H=256, W=256, C=36], HWC contiguous):
  - 2 row-blocks of 128 rows x 2 column-halves of 128 cols (4 work units,
    SBUF tiles [128, 128*36]; DMAs contiguous 18KB rows).
  - special (ch 0:3):  out = amax * tanh(alpha*(x+bias))
  - joint groups low(3:6) mid(6:15) high(15:36):
      soft_abs = sqrt((x+bias)^2 + eps)     (GPSIMD square when bias==0,
                                             else ACT Square; ACT Sqrt)
      chansum  = sum_c soft_abs             (DVE reduce over C)
      s        = 3x3 box(chansum), edge-replicated (DVE shifted adds; row
                 shifts via SBUF->SBUF DMA, halo between the row-blocks)
      G        = alpha*gss/(1 + s/p_sat)   (DVE, one reciprocal per block)
      v        = (x+bias) * G               (DVE scalar_tensor_tensor,
                                             G broadcast over C)
      out      = amax * tanh(v)             (ACT Tanh whole tile; amax via
                                             DVE ch 0:15 + ACT ch 15:36)
"""

import numpy as np

B, H, W, C = 8, 256, 256, 36
WH = W // 2          # column-half width
FD = W * C           # free elements per full row
FDH = WH * C         # free elements per half row
GROUPS = [("low", 3, 6), ("mid", 6, 15), ("high", 15, 36)]
N_CORES = 8

_NC_CACHE: dict[tuple, object] = {}


def _build(p: dict[str, float]):
    from contextlib import ExitStack

    import concourse.bass as bass  # noqa: F401
    import concourse.tile as tile
    from concourse import bacc, mybir

    f32 = mybir.dt.float32
    AF = mybir.ActivationFunctionType
    OP = mybir.AluOpType
    AX = mybir.AxisListType

    nc = bacc.Bacc(
        "TRN2", target_bir_lowering=False, debug=False, num_devices=N_CORES
    )
    x = nc.dram_tensor("x", [H, FD], f32, kind="ExternalInput").ap()
    bands = nc.dram_tensor("bands", [128, 256], f32,
                           kind="ExternalInput").ap()
    halos = nc.dram_tensor("halos", [1, 256], f32, kind="ExternalInput").ap()
    y = nc.dram_tensor("y", [H, FD], f32, kind="ExternalOutput").ap()

    joint_bias_zero = all(p[g + "_bias"] == 0.0 for g, _, _ in GROUPS)

    with tile.TileContext(nc) as tc, ExitStack() as ctx:
        xp = ctx.enter_context(tc.tile_pool(name="xp", bufs=4))
        sqp = ctx.enter_context(tc.tile_pool(name="sqp", bufs=3))
        mp = ctx.enter_context(tc.tile_pool(name="maps", bufs=1))
        cp = ctx.enter_context(tc.tile_pool(name="consts", bufs=1))
        psp = ctx.enter_context(tc.tile_pool(name="psum", bufs=3,
                                             space="PSUM"))

        _consts: dict[float, object] = {}

        def constant(val: float):
            """[128,1] SBUF tile holding `val` (for ACT bias operands)."""
            val = float(val)
            if val not in _consts:
                ct = cp.tile([128, 1], f32, tag=f"const{len(_consts)}")
                nc.vector.memset(ct[:], val)
                _consts[val] = ct
            return _consts[val][:]

        xt = {}
        cs = {}
        Tm = {}
        Sall = {}

        # Phase A: load, soft_abs, channel sums (per row-block x col-half)
        for rb in range(2):
            for g, _, _ in GROUPS:
                cs[(rb, g)] = mp.tile([128, W], f32, tag=f"cs{rb}{g}",
                                      name=f"cs{rb}{g}")
            for h in range(2):
                t = xp.tile([128, FDH], f32, tag="x")
                nc.sync.dma_start(
                    t[:], x[rb * 128 : (rb + 1) * 128,
                            h * FDH : (h + 1) * FDH])
                xt[(rb, h)] = t
                x3 = t[:].rearrange("p (w c) -> p w c", c=C)
                sq = sqp.tile([128, FDH], f32, tag="sq")
                sq3 = sq[:].rearrange("p (w c) -> p w c", c=C)
                if joint_bias_zero:
                    # one contiguous Square over ch 3:36 (GPSIMD would be
                    # faster in isolation but steals DVE SBUF ports)
                    nc.scalar.activation(
                        sq[:].rearrange("p (w c) -> p w c", c=C)[:, :, 3:36],
                        x3[:, :, 3:36], AF.Square)
                else:
                    for g, c0, c1 in GROUPS:
                        nc.scalar.activation(sq3[:, :, c0:c1],
                                             x3[:, :, c0:c1], AF.Square,
                                             bias=constant(p[g + "_bias"]))
                for g, c0, c1 in GROUPS:
                    nc.scalar.activation(sq3[:, :, c0:c1], sq3[:, :, c0:c1],
                                         AF.Sqrt,
                                         bias=constant(p[g + "_eps"]))
                for g, c0, c1 in GROUPS:
                    nc.vector.reduce_sum(
                        cs[(rb, g)][:, h * WH : (h + 1) * WH],
                        sq3[:, :, c0:c1], axis=AX.X)

            # W-direction 3-tap box with replicate edges (full width)
            for g, c0, c1 in GROUPS:
                c_ = cs[(rb, g)]
                T = mp.tile([128, W], f32, tag=f"T{rb}{g}")
                nc.vector.tensor_add(T[:, 1 : W - 1], c_[:, 0 : W - 2],
                                     c_[:, 2:W])
                nc.vector.tensor_add(T[:, 0:1], c_[:, 0:1], c_[:, 1:2])
                nc.vector.tensor_add(T[:, W - 1 : W], c_[:, W - 2 : W - 1],
                                     c_[:, W - 1 : W])
                nc.vector.tensor_add(T[:], T[:], c_[:])
                Tm[(rb, g)] = T

        # Phase A2: H-direction 3-tap box on TensorE: S = band.T @ T plus a
        # rank-1 halo matmul for the row from the other block (replicate
        # edges are baked into the band matrices, passed in as inputs).
        # PE rhs partition APs must start at 0/32/64/96, so block 1's halo
        # row (T0 row 127) is first DMA-copied to partition 0 of a scratch
        # tile. The 3 group maps of a block share one SBUF tile so one
        # reciprocal per block gives
        # G = 1/(s/(p_sat*alpha*gss) + 1/(alpha*gss)) = alpha*gss/(1+s/p_sat).
        bands_t = cp.tile([128, 256], f32, tag="bands")
        halos_t = cp.tile([1, 256], f32, tag="halos")
        nc.sync.dma_start(bands_t[:], bands[:])
        nc.sync.dma_start(halos_t[:], halos[:])
        hrow = {}
        for g, c0, c1 in GROUPS:
            hr = mp.tile([1, W], f32, tag=f"hrow{g}", name=f"hrow{g}")
            nc.gpsimd.dma_start(hr[:], Tm[(0, g)][127:128, :])
            hrow[g] = hr
        rscratch = mp.tile([128, 3 * W], f32, tag="rscratch")
        for rb in range(2):
            Sa = mp.tile([128, 3 * W], f32, tag=f"Sall{rb}")
            Sall[rb] = Sa
            for gi, (g, c0, c1) in enumerate(GROUPS):
                S_ps = psp.tile([128, W], f32, tag="ps")
                nc.tensor.matmul(S_ps[:],
                                 bands_t[:, rb * 128 : (rb + 1) * 128],
                                 Tm[(rb, g)][:], start=True, stop=False)
                halo_rhs = Tm[(1, g)][0:1, :] if rb == 0 else hrow[g][:]
                nc.tensor.matmul(S_ps[:],
                                 halos_t[0:1, rb * 128 : (rb + 1) * 128],
                                 halo_rhs, start=False, stop=True)
                S = Sa[:, gi * W : (gi + 1) * W]
                ags = p[g + "_alpha"] * p[g + "_gss"]
                nc.vector.tensor_scalar(S, S_ps[:],
                                        float(1.0 / (p[g + "_p_sat"] * ags)),
                                        float(1.0 / ags),
                                        op0=OP.mult, op1=OP.add)
            nc.vector.reciprocal_approx_accurate(Sa[:], Sa[:],
                                                 rscratch[:])

        # Phase B: v = (x+bias)/r in place, tanh, amax, store
        for rb in range(2):
            for h in range(2):
                t = xt[(rb, h)]
                x3 = t[:].rearrange("p (w c) -> p w c", c=C)
                sp = x3[:, :, 0:3]
                if p["special_alpha"] != 1.0 or p["special_bias"] != 0.0:
                    nc.scalar.activation(
                        sp, sp, AF.Copy,
                        bias=float(p["special_alpha"] * p["special_bias"]),
                        scale=float(p["special_alpha"]))
                for gi, (g, c0, c1) in enumerate(GROUPS):
                    cg = c1 - c0
                    rsl = Sall[rb][:, gi * W + h * WH : gi * W + (h + 1) * WH]
                    rb_ = rsl.unsqueeze(2).to_broadcast([128, WH, cg])
                    nc.vector.scalar_tensor_tensor(
                        x3[:, :, c0:c1], x3[:, :, c0:c1],
                        float(p[g + "_bias"]), rb_,
                        op0=OP.add, op1=OP.mult)
                nc.scalar.activation(t[:], t[:], AF.Tanh)
                # final amax scale (skipped when amax == 1):
                # DVE ch 0:6, ACT ch 6:36
                if p["special_amax"] != 1.0:
                    nc.vector.tensor_scalar_mul(x3[:, :, 0:3], x3[:, :, 0:3],
                                                float(p["special_amax"]))
                if p["low_amax"] != 1.0:
                    nc.vector.tensor_scalar_mul(x3[:, :, 3:6], x3[:, :, 3:6],
                                                float(p["low_amax"]))
                if p["mid_amax"] != 1.0:
                    nc.scalar.mul(x3[:, :, 6:15], x3[:, :, 6:15],
                                  float(p["mid_amax"]))
                if p["high_amax"] != 1.0:
                    nc.scalar.mul(x3[:, :, 15:36], x3[:, :, 15:36],
                                  float(p["high_amax"]))
                nc.sync.dma_start(
                    y[rb * 128 : (rb + 1) * 128, h * FDH : (h + 1) * FDH],
                    t[:])

    nc.compile()
    return nc


_SCALARS = [
    "special_bias", "special_alpha", "special_amax", "special_eps",
    "low_bias", "low_alpha", "low_amax", "low_eps", "low_gss", "low_p_sat",
    "mid_bias", "mid_alpha", "mid_amax", "mid_eps", "mid_gss", "mid_p_sat",
    "high_bias", "high_alpha", "high_amax", "high_eps", "high_gss",
    "high_p_sat",
]


def build_nc(**inputs):
    """Build (or fetch cached) compiled Bass program for these scalar params."""
    p = {k: float(np.asarray(inputs[k]).reshape(-1)[0]) for k in _SCALARS}
    key = tuple(p[k] for k in _SCALARS)
    if key not in _NC_CACHE:
        _NC_CACHE[key] = _build(p)
    return _NC_CACHE[key]


def _band_arrays():
    """Band matrices (lhsT, [k, m] = contribution of input row k to output
    row m) for the H-direction 3-tap box, replicate edges baked in, plus
    rank-1 halo row selectors."""
    A = np.zeros((128, 128), np.float32)
    for m in range(128):
        for k in (m - 1, m, m + 1):
            if 0 <= k < 128:
                A[k, m] = 1.0
    A0 = A.copy()
    A0[0, 0] = 2.0       # top replicate (block 0)
    A1 = A.copy()
    A1[127, 127] = 2.0   # bottom replicate (block 1)
    bands = np.concatenate([A0, A1], axis=1)          # [128, 256]
    halos = np.zeros((1, 256), np.float32)
    halos[0, 127] = 1.0      # block 0: out[127] += T1[0]
    halos[0, 128 + 0] = 1.0  # block 1: out[0]   += T0[127]
    return bands, halos


def kernel(**inputs) -> np.ndarray:
    from concourse.bass_utils import run_bass_kernel_spmd

    raw = np.ascontiguousarray(np.asarray(inputs["raw_coeffs"],
                                          dtype=np.float32))
    assert raw.shape == (B, H, W, C), raw.shape
    nc = build_nc(**inputs)
    bands, halos = _band_arrays()
    in_maps = [{"x": raw[i].reshape(H, FD), "bands": bands, "halos": halos}
               for i in range(N_CORES)]
    res = run_bass_kernel_spmd(nc, in_maps, list(range(N_CORES)))
    out = np.stack([res.results[i]["y"].reshape(H, W, C)
                    for i in range(N_CORES)])
    return out.astype(np.float32)
